# revision 37
# baseline (speedup 1.0000x reference)
"""Causal self-attention (separate heads) TRN2 Bass kernel.

Problem (hardcoded): B=4, T=2048, C=1024, H=16, HS=64, fp32.
  q/k/v = per-head linear projections of x; att = softmax(causal(q k^T / 8));
  y = att v; out = concat_heads(y) @ Wp.T + bp.

Sharding over 8 NeuronCores: core c -> batch b = c//2, head-group hg = c%2
(8 heads each). Each core computes a [T, C] partial of the output (its heads'
contribution through the column slice of Wp); host sums the two partials per
batch and adds bp + the bv-fold (softmax rows sum to 1, so the V bias passes
through attention untouched: contributes bv_flat @ Wp.T, added on host).

Bias folds: K needs no bias for the softmax (per-query logit constants
cancel); adding bq to Q alone yields exactly the softmax-equivalent logits
q.k + bq.k. V's bias is folded on the host (see above).

dtypes: everything bf16 except PSUM (fp32) and the logits path through exp
(PSUM fp32 -> bf16 eS). Tolerance 2e-2 allows it (measured ~2.7e-3); bf16
halves DMA, enables Fast Weight Load on stationary operands, and doubles
DVE throughput on 16-bit ops.

Per-core schedule (single interleaved PE stream, block-granular chaining):
  init:     consts, resident x (all 8 c-chunks stay in SBUF for the whole
            kernel -> QK projections accumulate the full C contraction in
            one PSUM bank; x is DMAed once, not once per pair)
  V+QK0:    V[t, dm] 2 super-chunk passes (progressive with the x DMAs),
            evacuated to bf16 V via ACT (sc0) / DVE add (sc1); QK for pair 0
            interleaved the same way.
  stages p=0..3: attention for pair p: S^T pair-of-heads matmuls (row groups
            0-1 / 2-3, concurrent in the PE array) into a 2-bank PSUM tile,
            ONE ACT exp (scale=1/8, max-free softmax) per (j,i) covering both
            heads -> bf16 eS; causal diagonal via bf16 tri-mask mul; AV with
            ones-column-augmented bf16 V giving the softmax denominator in
            row 64; tails per j: DVE copy psY->SBUF, DMA the d rows into a
            partition-spread [32,32] tile, DVE reciprocal at free-dim 32
            (a [1,512] DVE reciprocal costs 3.3us -- 8 cycles/elem on one
            lane), DMA back, gpsimd broadcast, bf16 DVE mul into YT.
            QK(p+1) interleaved as fine-grained (2-matmul) PE filler, since
            the exp slightly out-paces the PE's S+AV work per block.
  C phase:  out[t, e] = sum_p YT_p^T @ WpT_p (YT/wp bf16 stationary-FWL) ->
            DVE/ACT copy alternating -> bf16 DMA out on both HWDGE queues.
"""
import ml_dtypes
import numpy as np

from concourse import bacc, bass_utils, tile, mybir

B, T, C, H, HS = 4, 2048, 1024, 16, 64
NCORE = 8
NPAIR = 4
NCH = T // 512
NST = T // 128

f32 = mybir.dt.float32
f32r = mybir.dt.float32r
bf16 = mybir.dt.bfloat16
EXP = mybir.ActivationFunctionType.Exp
LN = mybir.ActivationFunctionType.Ln
COPY = mybir.ActivationFunctionType.Copy

_CACHE = {}


def _build():
    nc = bacc.Bacc(None, target_bir_lowering=False)

    xT = nc.declare_dram_parameter("xT", [C, T], bf16, isOutput=False)
    wq = nc.declare_dram_parameter("wq", [128, 8, 512], bf16, isOutput=False)
    wk = nc.declare_dram_parameter("wk", [128, 8, 512], bf16, isOutput=False)
    wv = nc.declare_dram_parameter("wv", [128, 8, 512], bf16, isOutput=False)
    wp = nc.declare_dram_parameter("wp", [128, 4, 1024], bf16, isOutput=False)
    bq = nc.declare_dram_parameter("bq", [128, 4], f32, isOutput=False)
    trimask = nc.declare_dram_parameter("trimask", [128, 128], bf16, isOutput=False)
    out = nc.declare_dram_parameter("out", [T, C], bf16, isOutput=True)

    with tile.TileContext(nc) as tc:
        with tc.tile_pool(name="persist", bufs=1) as pp:
            # ---- constants / persistent tensors ----
            # DMA issue order matters at the start: the first V matmuls need
            # wv[0:4] and x chunks 0-3, so those go first.
            tri2 = pp.tile([128, 2, 128], bf16, tag="tri2")
            bq_sb = pp.tile([128, 4], f32, tag="bq")
            V = pp.tile([128, NPAIR, NST, 130], bf16, tag="V")
            x_sb = pp.tile([128, 8, T], bf16, tag="x_sb")
            wv_sb = pp.tile([128, 8, 512], bf16, tag="wv_sb")

            # DMA issue costs ~650ns on the issuing queue; split across the
            # two HWDGE queues (SP + Activation, idle here) to halve it.
            for k in range(8):
                nc.scalar.dma_start(wv_sb[:, k, :], wv[:, k, :])
                nc.sync.dma_start(x_sb[:, k, :], xT[128 * k : 128 * k + 128, :])
            nc.scalar.dma_start(tri2[:, 0, :], trimask[:])
            nc.scalar.dma_start(tri2[:, 1, :], trimask[:])
            nc.scalar.dma_start(bq_sb[:], bq[:])

            onescol = pp.tile([128, 32], bf16, tag="onescol")
            nc.vector.memset(onescol[:], 1.0)
            for p in range(NPAIR):
                for i in range(NST):
                    nc.vector.tensor_copy(V[:, p, i, 64:130:65], onescol[:, 0:2])

            # PE emission-order chain, block granular
            _chain = {"prev": None, "first": None}

            def pe_mm(*args, **kw):
                inst = nc.tensor.matmul(*args, **kw)
                if _chain["first"] is None and _chain["prev"] is not None:
                    tile.add_dep_helper(
                        inst.ins, _chain["prev"].ins, sync=False,
                        reason="pe block order",
                    )
                if _chain["first"] is None:
                    _chain["first"] = inst
                _chain["prev"] = inst
                return inst

            def end_blk():
                _chain["first"] = None

            with tc.tile_pool(name="phBC", bufs=1) as pb:
              YT = pb.tile([128, NPAIR, T], bf16, tag="YT")
              with (
                  tc.tile_pool(name="wqk", bufs=2) as pwqk,
                  tc.tile_pool(name="qkt", bufs=2) as pqkt,
              ):
                qt_of = {}
                kt_of = {}

                def alloc_qkt(p):
                    qt_of[p] = pqkt.tile([128, T], bf16, tag="QTp", name="QTp")
                    kt_of[p] = pqkt.tile([128, T], bf16, tag="KTp", name="KTp")

                def qk_evac(p, proj, tch, sc, ps):
                    dest = qt_of[p] if proj == "q" else kt_of[p]
                    dslice = dest[:, 512 * tch : 512 * tch + 512]
                    if sc == 1:
                        nc.vector.tensor_add(dslice, ps[:], dslice)
                    elif proj == "q":
                        nc.vector.tensor_scalar_add(dslice, ps[:], bq_sb[:, p : p + 1])
                    else:
                        nc.vector.tensor_copy(dslice, ps[:])

                # ================= V + QK0 phase =================
                with (
                    tc.tile_pool(name="ps_V", bufs=4, space="PSUM") as psv,
                    tc.tile_pool(name="ps_qk0", bufs=2, space="PSUM") as pqk0,
                ):
                    alloc_qkt(0)
                    wq0 = pwqk.tile([128, 8, 128], bf16, tag="wq_sl", name="wq_sl")
                    wk0 = pwqk.tile([128, 8, 128], bf16, tag="wk_sl", name="wk_sl")
                    nc.sync.dma_start(wq0[:], wq[:, :, 0:128])
                    nc.sync.dma_start(wk0[:], wk[:, :, 0:128])

                    Vv = V.rearrange("s p i (two d) -> s p i two d", two=2)
                    w0_of = {"q": wq0, "k": wk0}
                    for sc in range(2):
                        for st in range(NST):
                            ps = psv.tile([128, 512], f32, tag="ps_v", name="ps_v")
                            for kk in range(4):
                                k_abs = 4 * sc + kk
                                pe_mm(
                                    ps[:],
                                    x_sb[:, k_abs, 128 * st : 128 * st + 128],
                                    wv_sb[:, k_abs, :],
                                    start=(kk == 0),
                                    stop=(kk == 3),
                                )
                            end_blk()
                            src = ps.rearrange("s (p two d) -> s p two d", p=4, two=2)
                            dst = Vv[:, :, st, :, 0:64]
                            if sc == 0:
                                nc.scalar.activation(dst, src, COPY)
                            else:
                                nc.vector.tensor_add(dst, src, dst)
                        # QK for pair 0 on the same super-chunk
                        for proj in ("q", "k"):
                            for tch in range(NCH):
                                ps = pqk0.tile([128, 512], f32, tag="pw", name="ps_qk0")
                                for kk in range(4):
                                    k_abs = 4 * sc + kk
                                    pe_mm(
                                        ps[:],
                                        w0_of[proj][:, k_abs, :],
                                        x_sb[:, k_abs, 512 * tch : 512 * tch + 512],
                                        start=(kk == 0),
                                        stop=(kk == 3),
                                    )
                                end_blk()
                                qk_evac(0, proj, tch, sc, ps)

                # wp loaded here (DMA engines idle during attention; C
                # phase then starts without waiting on it)
                wp_bf = pb.tile([128, 4, 1024], bf16, tag="wpbf")
                nc.sync.dma_start(wp_bf[:], wp[:])

                # ============ attention stages with QK interleave ============
                with (
                    tc.tile_pool(name="phB_es", bufs=14) as pes,
                    tc.tile_pool(name="phB_rep", bufs=2) as prep,
                    tc.tile_pool(name="ps_S", bufs=2, space="PSUM") as ppss,
                    tc.tile_pool(name="ps_work", bufs=1, space="PSUM") as pwork,
                    tc.tile_pool(name="ps_Y", bufs=1, space="PSUM") as psy,
                ):
                    # -- QK projection emission units for one pair --
                    def qk_units(p):
                        state = {}

                        def load_wslices():
                            alloc_qkt(p)
                            wq_sl = pwqk.tile(
                                [128, 8, 128], bf16, tag="wq_sl", name="wq_sl"
                            )
                            wk_sl = pwqk.tile(
                                [128, 8, 128], bf16, tag="wk_sl", name="wk_sl"
                            )
                            nc.sync.dma_start(
                                wq_sl[:], wq[:, :, 128 * p : 128 * p + 128]
                            )
                            nc.sync.dma_start(
                                wk_sl[:], wk[:, :, 128 * p : 128 * p + 128]
                            )
                            state["w"] = {"q": wq_sl, "k": wk_sl}

                        # Fine-grained steps (2 matmuls each, ~430ns) so the
                        # filler can slot into the ~200ns/block gap left when
                        # the ACT exp (slower per i than the PE's S+AV) back-
                        # pressures the S stream via the psS ring. The PSUM
                        # accumulation group of one (proj, tch) spans steps --
                        # has_written is per-bank, contiguity not required.
                        def step(proj, tch, kk0):
                            def go():
                                w_sl = state["w"][proj]
                                if kk0 == 0:
                                    state[(proj, tch)] = pwork.tile(
                                        [128, 512], f32, tag="pw", name="pw"
                                    )
                                ps = state[(proj, tch)]
                                for kk in (kk0, kk0 + 1):
                                    pe_mm(
                                        ps[:],
                                        w_sl[:, kk, :],
                                        x_sb[:, kk, 512 * tch : 512 * tch + 512],
                                        start=(kk == 0),
                                        stop=(kk == 7),
                                    )
                                end_blk()
                                if kk0 == 6:
                                    dest = qt_of[p] if proj == "q" else kt_of[p]
                                    dslice = dest[:, 512 * tch : 512 * tch + 512]
                                    if proj == "q":
                                        nc.vector.tensor_scalar_add(
                                            dslice, ps[:], bq_sb[:, p : p + 1]
                                        )
                                    else:
                                        nc.vector.tensor_copy(dslice, ps[:])
                            return go

                        units = [load_wslices]
                        for proj in ("q", "k"):
                            for tch in range(NCH):
                                for kk0 in (0, 2, 4, 6):
                                    units.append(step(proj, tch, kk0))
                        return units

                    # Tail scheme, per (j): evacuate psY -> SBUF (releases the
                    # PSUM banks), DMA the 2x512 denominators partition-spread
                    # into a [32, 32] tile (so the DVE iterative-divide
                    # reciprocal runs at free-dim 32, ~0.4us, instead of 3.3us
                    # per [1, 512] row), reciprocal, DMA back to row layout,
                    # then gpsimd broadcast + bf16 mul into YT.
                    def emit_tail(p, j, psY):
                        dT = prep.tile([32, 32], bf16, tag="dT", name="dT", bufs=4)
                        yus = []
                        for hh in range(2):
                            yu = prep.tile([65, 512], bf16, tag="yu", name="yu", bufs=6)
                            nc.vector.tensor_copy(yu[:], psY[hh][:, :])
                            nc.sync.dma_start(
                                dT[16 * hh : 16 * hh + 16, :], yu[64:65, :]
                            )
                            yus.append(yu)
                        dTr = prep.tile([32, 32], bf16, tag="dTr", name="dTr", bufs=4)
                        with nc.allow_low_precision(reason="1/d in bf16 is fine"):
                            nc.vector.reciprocal(dTr[:], dT[:])
                        drow = prep.tile([1, 2, 512], bf16, tag="drow", name="drow", bufs=4)
                        nc.sync.dma_start(drow[:], dTr[:])
                        for hh in range(2):
                            repc = prep.tile([64, 512], bf16, tag="repc", name="repc", bufs=4)
                            nc.gpsimd.partition_broadcast(repc[:], drow[0:1, hh, :])
                            nc.vector.tensor_mul(
                                YT[64 * hh : 64 * hh + 64, p, 512 * j : 512 * j + 512],
                                yus[hh][0:64, :],
                                repc[:],
                            )

                    # C-output units: stage 3 has no QK filler, so its PE
                    # idles ~16us under the exp stream while the whole C
                    # phase waits. A C chunk (m, e) only needs stage-3 tails
                    # for j = m//4, which finish progressively -- so 24 of
                    # the 32 C units run inside stage 3 on the free "pw"
                    # bank. Out-DMA stays on the sync queue (the ACT queue
                    # is the exp pacer).
                    def c_unit(m, e):
                        ps = pwork.tile([128, 512], f32, tag="pw", name="pw_c")
                        for cp in range(NPAIR):
                            pe_mm(
                                ps[:],
                                YT[:, cp, 128 * m : 128 * m + 128],
                                wp_bf[:, cp, 512 * e : 512 * e + 512],
                                start=(cp == 0),
                                stop=(cp == 3),
                            )
                        end_blk()
                        ob = prep.tile([128, 512], bf16, tag="ob3", name="ob3", bufs=4)
                        nc.vector.tensor_copy(ob[:], ps[:])
                        nc.sync.dma_start(
                            out[128 * m : 128 * m + 128, 512 * e : 512 * e + 512],
                            ob[:],
                        )

                    GI = 2  # i-steps per attention block

                    for stage in range(NPAIR):
                        p = stage
                        filler = qk_units(p + 1) if p + 1 < NPAIR else []
                        fidx = 0

                        blocks = []  # (j, [i...], last_of_chunk)
                        for j in range(NCH):
                            nst_j = 4 * j + 4
                            for i0 in range(0, nst_j, GI):
                                ii = list(range(i0, min(i0 + GI, nst_j)))
                                blocks.append((j, ii, i0 + GI >= nst_j))

                        nfill = len(filler)
                        nblk = len(blocks)

                        eS_store = {}
                        psY_of = {}
                        c_emitted = 0
                        for n in range(nblk + 3):
                            if p == 3 and n >= 7:
                                cwant = min(24, (3 * (n - 6)) // 2)
                                while c_emitted < cwant:
                                    k = c_emitted
                                    c_unit(k // 2, k % 2)
                                    c_emitted += 1
                            # AV block n-3 (deeper S->AV lookahead)
                            if n >= 3:
                                j, ii, last = blocks[n - 3]
                                psY = psY_of[j]
                                nst_j = 4 * j + 4
                                for i in ii:
                                    off = max(0, 128 * i - 512 * j)
                                    eS = eS_store.pop((j, i))
                                    for hh in range(2):
                                        pe_mm(
                                            psY[hh][:, off:512],
                                            V[:, p, i, 65 * hh : 65 * hh + 65],
                                            eS[:, hh, off:512],
                                            start=(i == 0),
                                            stop=(i == nst_j - 1),
                                        )
                                end_blk()
                                if last:
                                    emit_tail(p, j, psY)
                            # filler QK unit(s), front-loaded
                            want = min(nfill, ((n + 1) * nfill) // max(1, int(0.9 * nblk)))
                            while fidx < want:
                                filler[fidx]()
                                fidx += 1
                            # S block n
                            if n < nblk:
                                j, ii, last = blocks[n]
                                if j not in psY_of:
                                    psY_of[j] = [
                                        psy.tile([65, 512], f32, tag="psYa",
                                                 name="psYa", bufs=2),
                                        psy.tile([65, 512], f32, tag="psYb",
                                                 name="psYb", bufs=1),
                                    ]
                                acts = []
                                for i in ii:
                                    off = max(0, 128 * i - 512 * j)
                                    psS = ppss.tile(
                                        [128, 2, 512], f32, tag="psS", name="psS"
                                    )
                                    for hh in range(2):
                                        h0 = 64 * hh
                                        pe_mm(
                                            psS[:, hh, off:512],
                                            kt_of[p][h0 : h0 + 64, 128 * i : 128 * i + 128],
                                            qt_of[p][
                                                h0 : h0 + 64,
                                                512 * j + off : 512 * j + 512,
                                            ],
                                            start=True,
                                            stop=True,
                                        )
                                    eS = pes.tile([128, 2, 512], bf16, tag="eS", name="eS")
                                    acts.append((i, off, psS, eS))
                                    eS_store[(j, i)] = eS
                                end_blk()
                                for i, off, psS, eS in acts:
                                    nc.scalar.activation(
                                        eS[:, :, off:512], psS[:, :, off:512], EXP,
                                        scale=0.125,
                                    )
                                    if i >= 4 * j:
                                        nc.vector.tensor_mul(
                                            eS[:, :, off : off + 128],
                                            eS[:, :, off : off + 128],
                                            tri2[:],
                                        )
                        while fidx < nfill:
                            filler[fidx]()
                            fidx += 1

              # ================= C phase =================
              with (
                  tc.tile_pool(name="phC", bufs=4) as pc,
                  tc.tile_pool(name="ps_O", bufs=2, space="PSUM") as pso,
              ):
                  for m in range(12, NST):
                      for e in range(2):
                          ps = pso.tile([128, 512], f32, tag="psO", name="psO")
                          for p in range(NPAIR):
                              pe_mm(
                                  ps[:],
                                  YT[:, p, 128 * m : 128 * m + 128],
                                  wp_bf[:, p, 512 * e : 512 * e + 512],
                                  start=(p == 0),
                                  stop=(p == 3),
                              )
                          end_blk()
                          ob = pc.tile([128, 512], bf16, tag="ob", name="ob")
                          if (2 * m + e) % 2 == 0:
                              nc.vector.tensor_copy(ob[:], ps[:])
                          else:
                              nc.scalar.activation(ob[:], ps[:], COPY)
                          dma_eng = nc.sync if (2 * m + e) % 2 == 0 else nc.scalar
                          dma_eng.dma_start(
                              out[
                                  128 * m : 128 * m + 128,
                                  512 * e : 512 * e + 512,
                              ],
                              ob[:],
                          )

    nc.compile()
    return nc


def _prep_core_inputs(x, Wq, bq, Wk, Wv, Wp, core):
    b, hg = core // 2, core % 2
    h0 = 8 * hg

    def wprep(W):
        A = W[h0 : h0 + 8]
        Bm = np.transpose(A, (2, 0, 1)).reshape(C, 512)
        return np.ascontiguousarray(
            Bm.reshape(8, 128, 512).transpose(1, 0, 2).astype(ml_dtypes.bfloat16)
        )

    wp_sl = Wp[:, 512 * hg : 512 * hg + 512]
    wp_prep = np.ascontiguousarray(
        wp_sl.T.reshape(4, 128, 1024).transpose(1, 0, 2).astype(ml_dtypes.bfloat16)
    )

    return {
        "xT": np.ascontiguousarray(x[b].T.astype(ml_dtypes.bfloat16)),
        "wq": wprep(Wq),
        "wk": wprep(Wk),
        "wv": wprep(Wv),
        "wp": wp_prep,
        "bq": np.ascontiguousarray(bq[h0 : h0 + 8].reshape(4, 128).T),
        "trimask": np.ascontiguousarray(np.triu(np.ones((128, 128), ml_dtypes.bfloat16))),
    }


TRACE = False
TRACE_KW = {}


def kernel(x, Wq, bq, Wk, bk, Wv, bv, Wp, bp):
    x = np.asarray(x, np.float32)
    Wq = np.asarray(Wq, np.float32)
    bq = np.asarray(bq, np.float32)
    Wk = np.asarray(Wk, np.float32)
    bk = np.asarray(bk, np.float32)
    Wv = np.asarray(Wv, np.float32)
    bv = np.asarray(bv, np.float32)
    Wp = np.asarray(Wp, np.float32)
    bp = np.asarray(bp, np.float32)

    if "nc" not in _CACHE:
        _CACHE["nc"] = _build()
    nc = _CACHE["nc"]

    # bk cancels in the softmax given the Q-side fold (see module docstring);
    # bv passes through attention (rows of att sum to 1) into a constant
    # output offset bv_flat @ Wp.T, added here alongside bp.
    in_maps = [
        _prep_core_inputs(x, Wq, bq, Wk, Wv, Wp, core)
        for core in range(NCORE)
    ]
    res = bass_utils.run_bass_kernel_spmd(
        nc, in_maps, list(range(NCORE)), trace=TRACE, **TRACE_KW
    )
    _CACHE["last_result"] = res

    const = bp + bv.reshape(-1) @ Wp.T
    outp = np.empty((B, T, C), np.float32)
    for b in range(B):
        outp[b] = (
            res.results[2 * b]["out"].astype(np.float32)
            + res.results[2 * b + 1]["out"].astype(np.float32)
            + const
        )
    return outp
